# revision 2
# baseline (speedup 1.0000x reference)
"""Trainium2 Bass kernel for nn_DeepConvLSTM (B=8, T=20, 3-layer ConvLSTM, 5x5 SAME).

Sharding: data-parallel over batch across the 8 NeuronCores (one sample per
core, weights replicated, zero collectives). Per core, the 5x5 conv at each
timestep/layer is computed as 25 shift-accumulated matmuls into PSUM reading a
zero-padded [C,46,46] SBUF image; K is chunked by 128 channels (x-part /
h-part), M by gate (4 x 128 = i,f,g,o), N by output-row groups
(12,12,12,6 rows -> <=504 floats, one PSUM bank each). Layer 0's 2 input
channels are im2col'd on the host into a K=50 operand so its x-part is a
single matmul per (gate, n-chunk). LayerNorm stats use a ones-vector matmul
for the cross-partition reduction and a rank-1 matmul for the broadcast.

Conv matmul operands (weights, x, h) are bf16 (PE runs 1 cycle/row vs 4 for
fp32); PSUM accumulation, cell state, gates, LayerNorm, and the attention
epilogue stay fp32. Layer 2 keeps a flat fp32 copy of h for the output DMA.
"""

import numpy as np

B, T, CIN, HID, HW, NL = 8, 20, 2, 128, 42, 3
PW = HW + 4
NPIX = HW * HW
EPS = 1e-5
NCH = 4
CH_ROWS = [(0, 12), (12, 12), (24, 12), (36, 6)]
PS_OFF = [0, 512, 1024, 1536]
INV_N = 1.0 / (HID * NPIX)

_CACHE = {}


def _build(T_):
    import concourse.bacc as bacc
    import concourse.tile as tile
    from concourse import mybir
    from concourse.mybir import ActivationFunctionType as AF
    from concourse.mybir import AluOpType as ALU
    from contextlib import ExitStack

    f32 = mybir.dt.float32
    bf16 = mybir.dt.bfloat16
    nc = bacc.Bacc("TRN2", target_bir_lowering=False, debug=False)

    def din(name, shape, dt=f32):
        return nc.dram_tensor(name, shape, dt, kind="ExternalInput").ap()

    xe0 = din("xe0", (T_, 50, NPIX), bf16)
    wh0 = din("wh0", (HID, 25, 512), bf16)
    wx0 = din("wx0", (50, 512), bf16)
    w1d = din("w1", (HID, 2, 25, 512), bf16)
    w2d = din("w2", (HID, 2, 25, 512), bf16)
    lnw = din("lnw", (NL, HID, NPIX))
    lnb = din("lnb", (NL, HID, NPIX))
    pci = din("pci", (NL, HID, 1))
    pcf = din("pcf", (NL, HID, 1))
    pco = din("pco", (NL, HID, 1))
    cbd = din("cb", (NL, HID, 4))
    atw = din("attw", (HID, 1))
    atb = din("attb", (1, 1))
    y = nc.dram_tensor("y", (T_, HID, NPIX), f32, kind="ExternalOutput").ap()

    def v42(ap):
        return ap.rearrange("p (a b) -> p a b", a=HW)

    with tile.TileContext(nc) as tc:
        with ExitStack() as ctx:
            const = ctx.enter_context(tc.tile_pool(name="const", bufs=1))
            wpool = ctx.enter_context(tc.tile_pool(name="wts", bufs=1))
            state = ctx.enter_context(tc.tile_pool(name="state", bufs=1))
            work = ctx.enter_context(tc.tile_pool(name="work", bufs=1))
            small = ctx.enter_context(tc.tile_pool(name="small", bufs=1))
            psum = ctx.enter_context(tc.tile_pool(name="ps", bufs=2, space="PSUM"))
            dram = ctx.enter_context(tc.tile_pool(name="dram", bufs=1, space="DRAM"))

            ones_k = const.tile([128, 1], f32)
            nc.vector.memset(ones_k, 1.0)
            ones_b = const.tile([1, 128], f32)
            nc.vector.memset(ones_b, 1.0)
            eps_t = const.tile([1, 1], f32)
            nc.vector.memset(eps_t, EPS)
            attw_t = const.tile([128, 1], f32)
            nc.sync.dma_start(out=attw_t, in_=atw)
            attb_t = const.tile([1, 1], f32)
            nc.sync.dma_start(out=attb_t, in_=atb)

            hpad = state.tile([128, PW, PW], bf16, tag="hpad")
            nc.vector.memset(hpad, 0.0)
            c_t = state.tile([128, NPIX], f32, tag="c")
            hf32 = state.tile([128, NPIX], f32, tag="hf32")

            g_i = work.tile([128, NPIX], f32, tag="gi")
            g_f = work.tile([128, NPIX], f32, tag="gf")
            g_g = work.tile([128, NPIX], f32, tag="gg")
            scr = work.tile([128, NPIX], f32, tag="scr")
            hraw = work.tile([128, NPIX], f32, tag="hraw")

            stats = small.tile([128, 2], f32)
            sm = small.tile([1, 8], f32)
            sbs = small.tile([128, 2], f32)

            ia = dram.tile([T_, HID, NPIX], bf16, tag="ia")
            ib = dram.tile([T_, HID, NPIX], bf16, tag="ib")

            for l in range(NL):
                if l == 0:
                    w_h = wpool.tile([128, 25, 512], bf16, tag="w")
                    nc.sync.dma_start(out=w_h, in_=wh0)
                    w_x = wpool.tile([50, 512], bf16, tag="wx")
                    nc.sync.dma_start(out=w_x, in_=wx0)
                    w_t = None
                else:
                    w_t = wpool.tile([128, 2, 25, 512], bf16, tag="w")
                    src_w = w1d if l == 1 else w2d
                    nc.sync.dma_start(out=w_t[:, 0], in_=src_w[:, 0])
                    nc.sync.dma_start(out=w_t[:, 1], in_=src_w[:, 1])
                lnw_t = wpool.tile([128, NPIX], f32, tag="lnw")
                nc.sync.dma_start(out=lnw_t, in_=lnw[l])
                lnb_t = wpool.tile([128, NPIX], f32, tag="lnb")
                nc.sync.dma_start(out=lnb_t, in_=lnb[l])
                pci_t = wpool.tile([128, 1], f32, tag="pci")
                nc.sync.dma_start(out=pci_t, in_=pci[l])
                pcf_t = wpool.tile([128, 1], f32, tag="pcf")
                nc.sync.dma_start(out=pcf_t, in_=pcf[l])
                pco_t = wpool.tile([128, 1], f32, tag="pco")
                nc.sync.dma_start(out=pco_t, in_=pco[l])
                cb_t = wpool.tile([128, 4], f32, tag="cb")
                nc.sync.dma_start(out=cb_t, in_=cbd[l])

                nc.vector.memset(c_t, 0.0)

                src = (None, ia, ib)[l]
                dst = (ia, ib, None)[l]

                if l == 0:
                    xts = [state.tile([50, NPIX], bf16, tag=f"xin{j}", name=f"xe_l{l}_{j}") for j in range(2)]
                else:
                    xts = [state.tile([128, PW, PW], bf16, tag=f"xin{j}", name=f"xpad_l{l}_{j}") for j in range(2)]
                    nc.vector.memset(xts[0], 0.0)
                    nc.vector.memset(xts[1], 0.0)

                def load_x(t):
                    xt = xts[t % 2]
                    if l == 0:
                        nc.sync.dma_start(out=xt, in_=xe0[t])
                    else:
                        nc.sync.dma_start(
                            out=xt[:, 2 : 2 + HW, 2 : 2 + HW], in_=v42(src[t])
                        )

                for t in range(T_):
                    if t == 0:
                        load_x(0)
                    xt = xts[t % 2]
                    for m in range(4):
                        ps = psum.tile([128, 4 * 512], f32, tag="g")
                        msl = slice(m * 128, (m + 1) * 128)
                        last_xp = t == 0
                        if l == 0:
                            for n in range(NCH):
                                r0, nr = CH_ROWS[n]
                                fs, fsz = r0 * HW, nr * HW
                                nc.tensor.matmul(
                                    out=ps[:, PS_OFF[n] : PS_OFF[n] + fsz],
                                    lhsT=w_x[:, msl],
                                    rhs=xt[:, fs : fs + fsz],
                                    start=True,
                                    stop=last_xp,
                                )
                        else:
                            for s in range(25):
                                dy, dx = divmod(s, 5)
                                lhsT = w_t[:, 0, s, msl]
                                for n in range(NCH):
                                    r0, nr = CH_ROWS[n]
                                    nc.tensor.matmul(
                                        out=ps[:, PS_OFF[n] : PS_OFF[n] + nr * HW],
                                        lhsT=lhsT,
                                        rhs=xt[:, r0 + dy : r0 + dy + nr, dx : dx + HW],
                                        start=(s == 0),
                                        stop=(last_xp and s == 24),
                                    )
                        if m == 0 and t + 1 < T_:
                            load_x(t + 1)
                        if t > 0:
                            for s in range(25):
                                dy, dx = divmod(s, 5)
                                lhsT = w_h[:, s, msl] if l == 0 else w_t[:, 1, s, msl]
                                for n in range(NCH):
                                    r0, nr = CH_ROWS[n]
                                    nc.tensor.matmul(
                                        out=ps[:, PS_OFF[n] : PS_OFF[n] + nr * HW],
                                        lhsT=lhsT,
                                        rhs=hpad[
                                            :, r0 + dy : r0 + dy + nr, dx : dx + HW
                                        ],
                                        start=False,
                                        stop=(s == 24),
                                    )
                        if m != 2:
                            dst_g = (g_i, g_f, None, g_i)[m]
                            pp = (pci_t, pcf_t, None, pco_t)[m]
                            for n in range(NCH):
                                r0, nr = CH_ROWS[n]
                                fs, fsz = r0 * HW, nr * HW
                                nc.vector.scalar_tensor_tensor(
                                    out=dst_g[:, fs : fs + fsz],
                                    in0=c_t[:, fs : fs + fsz],
                                    scalar=pp[:, 0:1],
                                    in1=ps[:, PS_OFF[n] : PS_OFF[n] + fsz],
                                    op0=ALU.mult,
                                    op1=ALU.add,
                                )
                            nc.scalar.activation(
                                out=dst_g[:],
                                in_=dst_g[:],
                                func=AF.Sigmoid,
                                bias=cb_t[:, m : m + 1],
                            )
                        else:
                            for n in range(NCH):
                                r0, nr = CH_ROWS[n]
                                fs, fsz = r0 * HW, nr * HW
                                nc.scalar.activation(
                                    out=g_g[:, fs : fs + fsz],
                                    in_=ps[:, PS_OFF[n] : PS_OFF[n] + fsz],
                                    func=AF.Tanh,
                                    bias=cb_t[:, 2:3],
                                )
                            nc.vector.tensor_mul(scr[:], g_i[:], g_g[:])
                            nc.vector.tensor_mul(c_t[:], c_t[:], g_f[:])
                            nc.vector.tensor_add(c_t[:], c_t[:], scr[:])
                    # o is in g_i; tanh(c) -> g_f; hraw = o * tanh(c) with stats
                    nc.scalar.activation(out=g_f[:], in_=c_t[:], func=AF.Tanh)
                    nc.vector.tensor_mul(hraw[:], g_i[:], g_f[:])
                    nc.vector.reduce_sum(
                        out=stats[:, 0:1], in_=hraw[:], axis=mybir.AxisListType.X
                    )
                    nc.vector.tensor_mul(scr[:], hraw[:], hraw[:])
                    nc.vector.reduce_sum(
                        out=stats[:, 1:2], in_=scr[:], axis=mybir.AxisListType.X
                    )
                    ps_s = psum.tile([128, 4 * 512], f32, tag="g")
                    nc.tensor.matmul(
                        out=ps_s[0:1, 0:2],
                        lhsT=ones_k[:],
                        rhs=stats[:],
                        start=True,
                        stop=True,
                    )
                    nc.scalar.mul(sm[:, 0:1], ps_s[0:1, 0:1], INV_N)
                    nc.scalar.mul(sm[:, 1:2], ps_s[0:1, 1:2], INV_N)
                    nc.scalar.square(sm[:, 2:3], sm[:, 0:1])
                    nc.vector.tensor_sub(sm[:, 3:4], sm[:, 1:2], sm[:, 2:3])
                    nc.scalar.activation(
                        out=sm[:, 4:5], in_=sm[:, 3:4], func=AF.Sqrt, bias=eps_t[:]
                    )
                    nc.vector.reciprocal(sm[:, 5:6], sm[:, 4:5])
                    nc.vector.tensor_mul(sm[:, 6:7], sm[:, 0:1], sm[:, 5:6])
                    nc.scalar.mul(sm[:, 7:8], sm[:, 6:7], -1.0)
                    ps_b = psum.tile([128, 4 * 512], f32, tag="g")
                    nc.tensor.matmul(
                        out=ps_b[:, 0:2],
                        lhsT=ones_b[:],
                        rhs=sm[0:1, 5:8:2],
                        start=True,
                        stop=True,
                    )
                    nc.vector.tensor_copy(sbs[:], ps_b[:, 0:2])
                    nc.vector.tensor_scalar(
                        out=hraw[:],
                        in0=hraw[:],
                        scalar1=sbs[:, 0:1],
                        scalar2=sbs[:, 1:2],
                        op0=ALU.mult,
                        op1=ALU.add,
                    )
                    nc.vector.tensor_mul(scr[:], hraw[:], lnw_t[:])
                    if l < 2:
                        nc.vector.tensor_add(
                            hpad[:, 2 : 2 + HW, 2 : 2 + HW], v42(scr[:]), v42(lnb_t[:])
                        )
                        nc.sync.dma_start(
                            out=v42(dst[t]), in_=hpad[:, 2 : 2 + HW, 2 : 2 + HW]
                        )
                    else:
                        nc.vector.tensor_add(hf32[:], scr[:], lnb_t[:])
                        if t < T_ - 1:
                            nc.vector.tensor_copy(
                                hpad[:, 2 : 2 + HW, 2 : 2 + HW], v42(hf32[:])
                            )
                            nc.sync.dma_start(out=y[t], in_=hf32[:])

            # attention on the final h (flat f32 in hf32)
            ps_a = psum.tile([128, 4 * 512], f32, tag="g")
            for n in range(NCH):
                r0, nr = CH_ROWS[n]
                fs, fsz = r0 * HW, nr * HW
                nc.tensor.matmul(
                    out=ps_a[0:1, PS_OFF[n] : PS_OFF[n] + fsz],
                    lhsT=attw_t[:],
                    rhs=hf32[:, fs : fs + fsz],
                    start=True,
                    stop=True,
                )
            for n in range(NCH):
                r0, nr = CH_ROWS[n]
                fs, fsz = r0 * HW, nr * HW
                nc.scalar.activation(
                    out=g_g[0:1, fs : fs + fsz],
                    in_=ps_a[0:1, PS_OFF[n] : PS_OFF[n] + fsz],
                    func=AF.Sigmoid,
                    bias=attb_t[:],
                )
            ps_c = psum.tile([128, 4 * 512], f32, tag="g")
            for n in range(NCH):
                r0, nr = CH_ROWS[n]
                fs, fsz = r0 * HW, nr * HW
                nc.tensor.matmul(
                    out=ps_c[:, PS_OFF[n] : PS_OFF[n] + fsz],
                    lhsT=ones_b[:],
                    rhs=g_g[0:1, fs : fs + fsz],
                    start=True,
                    stop=True,
                )
            for n in range(NCH):
                r0, nr = CH_ROWS[n]
                fs, fsz = r0 * HW, nr * HW
                nc.vector.tensor_mul(
                    scr[:, fs : fs + fsz],
                    hf32[:, fs : fs + fsz],
                    ps_c[:, PS_OFF[n] : PS_OFF[n] + fsz],
                )
            nc.sync.dma_start(out=y[T_ - 1], in_=scr[:])

    nc.compile()
    return nc


def _pack_inputs(
    x,
    conv_w0,
    conv_b0,
    conv_w1,
    conv_b1,
    conv_w2,
    conv_b2,
    ln_w,
    ln_b,
    w_ci,
    w_cf,
    w_co,
    attn_w,
    attn_b,
):
    import ml_dtypes

    f = np.float32
    bf = ml_dtypes.bfloat16
    x = np.asarray(x, f)
    Bx, T_ = x.shape[0], x.shape[1]
    xp = np.pad(x, ((0, 0), (0, 0), (0, 0), (2, 2), (2, 2)))
    xe = np.empty((Bx, T_, 50, HW, HW), f)
    for dy in range(5):
        for dx in range(5):
            s = dy * 5 + dx
            xe[:, :, 2 * s : 2 * s + 2] = xp[:, :, :, dy : dy + HW, dx : dx + HW]
    xe = np.ascontiguousarray(xe.reshape(Bx, T_, 50, NPIX)).astype(bf)

    w0 = np.asarray(conv_w0, f).reshape(512, CIN + HID, 25)
    wh0 = np.ascontiguousarray(np.transpose(w0[:, CIN:], (1, 2, 0))).astype(bf)
    wx0 = (
        np.ascontiguousarray(np.transpose(w0[:, :CIN], (2, 1, 0)))
        .reshape(50, 512)
        .astype(bf)
    )

    def packw(w):
        w = np.asarray(w, f).reshape(512, 2, 128, 25)
        return np.ascontiguousarray(np.transpose(w, (2, 1, 3, 0))).astype(bf)

    shared = dict(
        wh0=wh0,
        wx0=wx0,
        w1=packw(conv_w1),
        w2=packw(conv_w2),
        lnw=np.asarray(ln_w, f).reshape(NL, HID, NPIX),
        lnb=np.asarray(ln_b, f).reshape(NL, HID, NPIX),
        pci=np.asarray(w_ci, f).reshape(NL, HID, 1),
        pcf=np.asarray(w_cf, f).reshape(NL, HID, 1),
        pco=np.asarray(w_co, f).reshape(NL, HID, 1),
        cb=np.stack(
            [np.asarray(b, f).reshape(4, 128).T for b in (conv_b0, conv_b1, conv_b2)]
        ),
        attw=np.ascontiguousarray(np.asarray(attn_w, f).reshape(1, HID).T),
        attb=np.asarray(attn_b, f).reshape(1, 1),
    )
    return [dict(shared, xe0=np.ascontiguousarray(xe[b])) for b in range(Bx)], T_


def kernel(**inputs):
    from concourse import bass_utils

    in_maps, T_ = _pack_inputs(**inputs)
    assert len(in_maps) == 8, "expected batch 8 mapped to 8 cores"
    if T_ not in _CACHE:
        _CACHE[T_] = _build(T_)
    nc = _CACHE[T_]
    res = bass_utils.run_bass_kernel_spmd(nc, in_maps, core_ids=list(range(8)))
    out = np.stack(
        [res.results[b]["y"].reshape(T_, HID, HW, HW) for b in range(8)]
    )
    return out.astype(np.float32)
